# revision 19
# baseline (speedup 1.0000x reference)
"""Trainium2 Bass kernel for nn_CIFAR10_Monarch_MLP2 (4-layer Monarch MLP + log_softmax).

Strategy
--------
Data-parallel over 8 NeuronCores: each core computes 2048 rows of the
16384-row batch with replicated weights; outputs are concatenated on host.

Per core, activations are kept feature-major ([feature partitions, batch
free]).  x is transposed to feature-major and cast to bf16 on the HOST, so
the device pipeline is load -> matmul with no casts or on-device transposes.
Batch tiles are NB=512 (full PSUM bank), halving instruction count vs 256.

The monarch permutation of layers 1/2 is folded into a host-side
re-arrangement of the weights:

 * w1 rows of block k are regrouped by destination plane l, each group padded
   to a fixed `chunk` (multiple of 32).  mm1 then runs natural M=128 tiles
   and its PSUM evictions scatter fragments into the plane layout with DVE
   copies, using the hardware's partition-shift capability (64-sized copies
   between halves, 32-sized between quadrants — HW-verified).
 * w2 columns are permuted to match the resulting plane-row order (pad rows
   get zero columns), so no data movement is needed for the permutation.

Layers 3/4 are small enough that their monarch pair (block-diag @ perm @
block-diag) is composed HOST-SIDE into a single dense matrix: PE cost is
instructions x N regardless of row/col utilization, and the dense form needs
fewer instructions (L3: 8 vs 12, L4: 4 vs 24) and no intermediate staging.
L4's bias is folded in as an extra contraction row (h4 row 127 := 1, W4c row
127 := bias), and its matmul is operand-swapped (activations stationary) so
logits land batch-major where log_softmax is a cheap free-dim reduction.
"""

import numpy as np
import ml_dtypes

import concourse.bass as bass
from concourse import bacc
import concourse.mybir as mybir
import concourse.tile as tile
from concourse.bass_utils import run_bass_kernel_spmd

F32 = mybir.dt.float32

# matmul operand dtype knob: mybir.dt.bfloat16 | float32r | float32
MM_DT = mybir.dt.bfloat16

N_CORES = 8
BATCH = 16384
B_CORE = BATCH // N_CORES  # 2048
NB = 512  # batch-tile free size (PSUM bank = 2KB -> 512 fp32)

# (p_dim, q_dim, chunk, s_dim) for monarch layers 1-2
LAYER_CFG = [
    (768, 750, 192, 750),
    (750, 250, 64, 250),
]


def _np_mmdt():
    return {
        mybir.dt.bfloat16: ml_dtypes.bfloat16,
        mybir.dt.float32r: np.float32,
        mybir.dt.float32: np.float32,
    }[MM_DT]


def arrange_layer(w1, w2, q_dim, chunk):
    """w1:(4,q,p), w2:(4,s,r=q) -> w1t:[4,p,QPAD] (mm1 lhsT), w2t:[4,QPAD,s]
    (mm2 lhsT), with the monarch permutation folded in (see module doc)."""
    nb, _, p_dim = w1.shape
    s_dim = w2.shape[1]
    QPAD = 4 * chunk
    w1t = np.zeros((nb, p_dim, QPAD), np.float32)
    w2t = np.zeros((nb, QPAD, s_dim), np.float32)
    for k in range(nb):
        for l in range(nb):
            qs = [q for q in range(q_dim) if (k * q_dim + q) % 4 == l]
            w1t[k, :, l * chunk : l * chunk + len(qs)] = w1[k, qs, :].T
            rs = [(k * q_dim + q) // 4 for q in qs]
            w2t[l, k * chunk : k * chunk + len(qs), :] = w2[l, :, rs]
    return w1t, w2t


def compose_monarch(w1, w2, out_features):
    """Dense [in_features, out_features] equivalent of one monarch linear."""
    nb, q_dim, p_dim = w1.shape
    _, s_dim, _ = w2.shape
    fin = nb * p_dim
    dense = np.zeros((fin, nb * s_dim), np.float64)
    w1d = w1.astype(np.float64)
    w2d = w2.astype(np.float64)
    for k in range(nb):
        for q in range(q_dim):
            f = k * q_dim + q
            l, r = f % 4, f // 4
            # out[l*s_dim + s] += w2[l, s, r] * (w1[k, q, :] . x[k*p_dim:...])
            dense[k * p_dim : (k + 1) * p_dim, l * s_dim : (l + 1) * s_dim] += (
                np.outer(w1d[k, q, :], w2d[l, :, r])
            )
    return dense[:, :out_features].astype(np.float32)


def evict_frags(k, m, chunk):
    """Fragments to scatter mm1's natural PSUM M-tile m of block k (padded
    rows [128m, 128m+128)) into the plane layout.

    Returns [(src_part0, size, plane_l, plane_tile, dst_part_base), ...].
    Fragment boundaries lie on the src 128-grid, dst 128-grid and l-chunk
    grid; shifted fragments are split to the DVE-legal 64 (or 32) grain.
    """
    grain = 64 if chunk % 64 == 0 else 32
    frags = []
    g = 128 * m
    end = 128 * (m + 1)
    while g < end:
        l = g // chunk
        dst = k * chunk + (g - l * chunk)  # global row within plane l
        # next boundary: chunk end, src tile end, dst tile end
        nb_ = min(end, (l + 1) * chunk, g + (128 - dst % 128))
        size = nb_ - g
        src_b = g - 128 * m
        dst_b = dst % 128
        if src_b % 128 == dst_b:
            frags.append((src_b, size, l, dst // 128, dst_b))
            g = nb_
        else:
            # shifted: emit at grain granularity (64: halves; 32: quadrants)
            step = min(grain, size)
            frags.append((src_b, step, l, dst // 128, dst_b))
            g += step
    return frags


def ktiles(p_dim):
    """[(row0, size), ...] 128-partition contraction tiles covering p_dim."""
    return [(r, min(128, p_dim - r)) for r in range(0, p_dim, 128)]


def prepare_weights(inputs):
    """Host-side arrangement of all weights/biases into DRAM-parameter arrays."""
    npdt = _np_mmdt()
    arrs = {}
    for li, (p_dim, q_dim, chunk, s_dim) in enumerate(LAYER_CFG, 1):
        w1 = np.asarray(inputs[f"w1_{li}"], np.float32)
        w2 = np.asarray(inputs[f"w2_{li}"], np.float32)
        w1t, w2t = arrange_layer(w1, w2, q_dim, chunk)
        arrs[f"w1t_{li}"] = w1t.astype(npdt)
        arrs[f"w2t_{li}"] = w2t.astype(npdt)
        bias = np.asarray(inputs[f"b{li}"], np.float32)  # [4*s_dim], f'=l*s+s
        # bias columns per (plane l, s-tile mt): [128, ncols]
        mts = ktiles(s_dim)
        cols = np.zeros((128, 4 * len(mts)), np.float32)
        for l in range(4):
            for mi, (m0, msz) in enumerate(mts):
                cols[:msz, l * len(mts) + mi] = bias[l * s_dim + m0 : l * s_dim + m0 + msz]
        arrs[f"bias_{li}"] = cols

    # ---- L3: dense composite [1000, 100], rows arranged to h3's tile layout
    w3c = compose_monarch(
        np.asarray(inputs["w1_3"], np.float32),
        np.asarray(inputs["w2_3"], np.float32), 100)
    h3_tiles = []  # (l2, m0, msz) in h3 storage order
    for l2 in range(4):
        for (m0, msz) in ktiles(250):
            h3_tiles.append((l2, m0, msz))
    # M padded to 128: rows 100..126 of h4 become relu(0)=0, and row 127
    # becomes relu(0 + bias=1) = 1 — the ones row for L4's folded bias.
    w3a = np.zeros((len(h3_tiles), 128, 128), np.float32)
    for ti, (l2, m0, msz) in enumerate(h3_tiles):
        w3a[ti, :msz, :100] = w3c[l2 * 250 + m0 : l2 * 250 + m0 + msz, :]
    arrs["w3a"] = w3a.astype(npdt)
    b3 = np.asarray(inputs["b3"], np.float32)
    b3c = np.pad(b3, (0, 28)).reshape(128, 1).astype(np.float32)
    b3c[127, 0] = 1.0
    arrs["bias_3"] = b3c

    # ---- L4: dense composite [100, 12] + bias folded at contraction row 127
    w4c = compose_monarch(
        np.asarray(inputs["w1_4"], np.float32),
        np.asarray(inputs["w2_4"], np.float32), 12)
    w4a = np.zeros((128, 12), np.float32)
    w4a[:100, :] = w4c
    b4 = np.asarray(inputs["b4"], np.float32)
    w4a[127, :10] = b4
    arrs["w4a"] = w4a.astype(npdt)
    return arrs


def build_nc(b_core=B_CORE, repeat=1, probe_mm1=False):
    """Build the single-core Bass program (SPMD: same program, per-core xT).
    repeat>1 re-runs the whole batch pipeline (for timing-by-differencing).
    probe_mm1 doubles L1 mm1's accumulation (identical result, pure extra PE
    work) to measure the marginal cost per matmul instruction."""
    nc = bacc.Bacc(None, target_bir_lowering=False)
    x_d = nc.declare_dram_parameter("xT", [3072, b_core], MM_DT, isOutput=False)
    y_d = nc.declare_dram_parameter("y", [b_core, 10], F32, isOutput=True)

    wd = {}
    for li, (p_dim, q_dim, chunk, s_dim) in enumerate(LAYER_CFG, 1):
        QPAD = 4 * chunk
        wd[f"w1t_{li}"] = nc.declare_dram_parameter(
            f"w1t_{li}", [4, p_dim, QPAD], MM_DT, isOutput=False)
        wd[f"w2t_{li}"] = nc.declare_dram_parameter(
            f"w2t_{li}", [4, QPAD, s_dim], MM_DT, isOutput=False)
        nmt = len(ktiles(s_dim))
        wd[f"bias_{li}"] = nc.declare_dram_parameter(
            f"bias_{li}", [128, 4 * nmt], F32, isOutput=False)
    wd["w3a"] = nc.declare_dram_parameter("w3a", [8, 128, 128], MM_DT, isOutput=False)
    wd["bias_3"] = nc.declare_dram_parameter("bias_3", [128, 1], F32, isOutput=False)
    wd["w4a"] = nc.declare_dram_parameter("w4a", [128, 12], MM_DT, isOutput=False)

    n_bt = b_core // NB

    with tile.TileContext(nc) as tc:
        with (
            tc.tile_pool(name="const", bufs=1) as const,
            tc.tile_pool(name="xT", bufs=8) as xTp,
            tc.tile_pool(name="acts", bufs=1) as acts,
            tc.tile_pool(name="h4p", bufs=1) as h4p,
            tc.tile_pool(name="psum_mm", bufs=7, space="PSUM") as psum_mm,
            tc.tile_pool(name="psum_s", bufs=1, space="PSUM") as psum_s,
            tc.tile_pool(name="sm", bufs=2) as smp,
        ):
            # ---- resident constants ----
            w1sb, w2sb, biassb = {}, {}, {}
            for li, (p_dim, q_dim, chunk, s_dim) in enumerate(LAYER_CFG, 1):
                QPAD = 4 * chunk
                kts = ktiles(p_dim)
                w1sb[li] = const.tile([128, len(kts) * 4 * QPAD], MM_DT, name=f"w1sb{li}", tag=f"w1sb{li}")
                for k in range(4):
                    for ki, (k0, ksz) in enumerate(kts):
                        col = (k * len(kts) + ki) * QPAD
                        nc.gpsimd.dma_start(
                            w1sb[li][:ksz, col : col + QPAD],
                            wd[f"w1t_{li}"][k, k0 : k0 + ksz, :],
                        )
                nrt = QPAD // 128
                w2sb[li] = const.tile([128, 4 * nrt * s_dim], MM_DT, name=f"w2sb{li}", tag=f"w2sb{li}")
                for l in range(4):
                    for rt in range(nrt):
                        col = (l * nrt + rt) * s_dim
                        # ACT HWDGE queue: streams in parallel with w1 on
                        # gpsimd so the first tile's mm2 isn't starved
                        nc.scalar.dma_start(
                            w2sb[li][:, col : col + s_dim],
                            wd[f"w2t_{li}"][l, 128 * rt : 128 * (rt + 1), :],
                        )
                nmt = len(ktiles(s_dim))
                biassb[li] = const.tile([128, 4 * nmt], F32, name=f"biassb{li}", tag=f"biassb{li}")
                nc.gpsimd.dma_start(biassb[li][:], wd[f"bias_{li}"][:, :])
            w3sb = const.tile([128, 8 * 128], MM_DT, name="w3sb", tag="w3sb")
            for ti in range(8):
                nc.gpsimd.dma_start(w3sb[:, ti * 128 : (ti + 1) * 128], wd["w3a"][ti, :, :])
            biassb3 = const.tile([128, 1], F32, name="biassb3", tag="biassb3")
            nc.gpsimd.dma_start(biassb3[:], wd["bias_3"][:, :])
            w4sb = const.tile([128, 12], MM_DT, name="w4sb", tag="w4sb")
            nc.gpsimd.dma_start(w4sb[:], wd["w4a"][:, :])

            # h4: [128, NB]; rewritten fully each batch-tile by the L3 evict
            # (rows 100..126 = 0, row 127 = 1 via the padded w3a/bias_3)
            h4 = h4p.tile([128, NB], MM_DT, name="h4", tag="h4")

            # ---- batch-tile pipeline ----
            for bt in [t for _ in range(repeat) for t in range(n_bt)]:
                # xT tiles: per block k, [128, 6*NB] feature-major bf16,
                # loaded straight from the host-transposed x.
                xk = []
                for k in range(4):
                    xt = xTp.tile([128, 6 * NB], MM_DT, name=f"x{k}", tag="xt")
                    src = x_d[768 * k : 768 * (k + 1), bt * NB : (bt + 1) * NB]
                    nc.sync.dma_start(
                        xt[:].rearrange("p (g c) -> p g c", g=6),
                        src.rearrange("(g p) c -> p g c", p=128),
                    )
                    xk.append(xt)
                h = None

                for li, (p_dim, q_dim, chunk, s_dim) in enumerate(LAYER_CFG, 1):
                    QPAD = 4 * chunk
                    ntl = QPAD // 128  # plane tiles
                    kts = ktiles(p_dim)
                    nkt = len(kts)
                    # --- mm1: natural block M-tiles (M=128, no col splits);
                    # evictions scatter to plane layout via (possibly
                    # partition-shifted) DVE fragment copies.  Legal shifts:
                    # any size at shift 0; 64-sized between halves; 32-sized
                    # between quadrants (HW-verified quadrant routing).
                    planes = acts.tile([128, 4 * ntl * NB], MM_DT, name=f"planes{li}", tag=f"planes{li}")
                    for k in range(4):
                        for m in range(ntl):
                            ps = psum_mm.tile([128, NB], F32, name="ps_mm", tag="ps_mm")
                            for rep in range(2 if (probe_mm1 and li == 1) else 1):
                                for ki, (k0, ksz) in enumerate(kts):
                                    if li == 1:
                                        rhs = xk[k][:, ki * NB : (ki + 1) * NB]
                                    else:
                                        hcol = in_tiles[k][ki][0]
                                        rhs = h[:ksz, hcol : hcol + NB]
                                    wcol = (k * nkt + ki) * QPAD + 128 * m
                                    nc.tensor.matmul(
                                        ps[:, :],
                                        w1sb[li][:ksz, wcol : wcol + 128],
                                        rhs,
                                        start=(ki == 0),
                                        stop=(ki == nkt - 1),
                                    )
                            for (s0, sz, l, jt, db) in evict_frags(k, m, chunk):
                                pcol = (l * ntl + jt) * NB
                                dst = planes[db : db + sz, pcol : pcol + NB]
                                src = ps[s0 : s0 + sz, :]
                                # DVE: the only engine that can both read PSUM
                                # and shift partitions (gpsimd can't touch
                                # PSUM; ACT would thrash activation tables)
                                nc.vector.tensor_copy(dst, src)

                    # --- mm2: planes -> next-layer blocks (relu+bias on evict)
                    mts = ktiles(s_dim)
                    nmt = len(mts)
                    hn = acts.tile([128, 4 * nmt * NB], MM_DT, name=f"h{li + 1}", tag=f"h{li + 1}")
                    for l in range(4):
                        for mi, (m0, msz) in enumerate(mts):
                            ps = psum_mm.tile([128, NB], F32, name="ps_mm", tag="ps_mm")
                            for rt in range(ntl):
                                wcol = (l * ntl + rt) * s_dim + m0
                                nc.tensor.matmul(
                                    ps[:msz, :],
                                    w2sb[li][:, wcol : wcol + msz],
                                    planes[:, (l * ntl + rt) * NB : (l * ntl + rt + 1) * NB],
                                    start=(rt == 0),
                                    stop=(rt == ntl - 1),
                                )
                            hcol = (l * nmt + mi) * NB
                            nc.scalar.activation(
                                hn[:msz, hcol : hcol + NB],
                                ps[:msz, :],
                                mybir.ActivationFunctionType.Relu,
                                bias=biassb[li][:msz, l * nmt + mi : l * nmt + mi + 1],
                            )
                    in_tiles = [
                        [((l * nmt + mi) * NB, msz) for mi, (m0, msz) in enumerate(mts)]
                        for l in range(4)
                    ]
                    h = hn

                # ---- L3: dense composite 1000 -> 100 (8 K-tiles, 1 M-tile)
                ps3 = psum_mm.tile([128, NB], F32, name="ps_mm", tag="ps_mm")
                for ti in range(8):
                    l2, ki = ti // 2, ti % 2
                    hcol, ksz = in_tiles[l2][ki]
                    nc.tensor.matmul(
                        ps3[:, :],
                        w3sb[:ksz, ti * 128 : ti * 128 + 128],
                        h[:ksz, hcol : hcol + NB],
                        start=(ti == 0),
                        stop=(ti == 7),
                    )
                nc.scalar.activation(
                    h4[:, :], ps3[:, :], mybir.ActivationFunctionType.Relu,
                    bias=biassb3[:, 0:1])

                # ---- L4: operand-swapped dense composite (bias via row 127):
                # logits[b, o] = sum_f h4[f, b] * w4a[f, o]
                for sub in range(NB // 128):
                    ps4 = psum_s.tile([128, 12], F32, name="ps4", tag="ps4")
                    nc.tensor.matmul(
                        ps4[:, :],
                        h4[:, sub * 128 : (sub + 1) * 128],
                        w4sb[:, :],
                        start=True,
                        stop=True,
                    )
                    # log_softmax over cols 0..9 (f32).  ln(s) is computed as
                    # bit-trick initial guess + one Newton step (2 ACT Exp ops)
                    # so the ACT engine only ever needs the exp table set —
                    # a Ln op would force a ~2.7us table-set switch per chain.
                    sm = smp.tile([128, 10], F32, name="sm", tag="sm")
                    nc.vector.tensor_copy(sm[:], ps4[:, 0:10])
                    mx = smp.tile([128, 1], F32, name="mx", tag="mx")
                    nc.vector.reduce_max(mx[:], sm[:], axis=mybir.AxisListType.X)
                    nmx = smp.tile([128, 1], F32, name="nmx", tag="nmx")
                    nc.vector.tensor_scalar_mul(nmx[:], mx[:], -1.0)
                    ex = smp.tile([128, 10], F32, name="ex", tag="ex")
                    nc.scalar.activation(
                        ex[:], sm[:], mybir.ActivationFunctionType.Exp, bias=nmx[:])
                    sme = smp.tile([128, 1], F32, name="sme", tag="sme")
                    nc.vector.reduce_sum(sme[:], ex[:], axis=mybir.AxisListType.X)
                    # y0 = bits(s)*ln2/2^23 - (127*ln2 - 0.0298): |y0-ln s|<=.03
                    smi = smp.tile([128, 1], F32, name="smi", tag="smi")
                    nc.vector.tensor_copy(smi[:], sme[:].bitcast(mybir.dt.int32))
                    y0 = smp.tile([128, 1], F32, name="y0", tag="y0")
                    nc.vector.tensor_scalar(
                        y0[:], smi[:], 8.2629582e-8, 87.9998919,
                        mybir.AluOpType.mult, mybir.AluOpType.subtract)
                    e0 = smp.tile([128, 1], F32, name="e0", tag="e0")
                    nc.scalar.activation(
                        e0[:], y0[:], mybir.ActivationFunctionType.Exp,
                        bias=0.0, scale=-1.0)
                    # ofs = nmx - ln(s); ln(s) ~= y0 + s*exp(-y0) - 1
                    se = smp.tile([128, 1], F32, name="se", tag="se")
                    nc.vector.tensor_mul(se[:], sme[:], e0[:])
                    t1 = smp.tile([128, 1], F32, name="t1", tag="t1")
                    nc.vector.tensor_sub(t1[:], nmx[:], y0[:])
                    t2 = smp.tile([128, 1], F32, name="t2", tag="t2")
                    nc.vector.tensor_sub(t2[:], t1[:], se[:])
                    ofs = smp.tile([128, 1], F32, name="ofs", tag="ofs")
                    nc.vector.tensor_scalar_add(ofs[:], t2[:], 1.0)
                    out_t = smp.tile([128, 10], F32, name="out_t", tag="out_t")
                    nc.vector.tensor_scalar_add(out_t[:], sm[:], ofs[:])
                    row0 = bt * NB + sub * 128
                    nc.sync.dma_start(y_d[row0 : row0 + 128, :], out_t[:])
    nc.compile()
    return nc


def make_in_maps(inputs):
    """Per-core input maps: host-transposed bf16 x shards + shared weights."""
    npdt = _np_mmdt()
    x = np.asarray(inputs["x"], np.float32)
    xT = x.T.astype(npdt)  # [3072, 16384] feature-major
    arrs = prepare_weights(inputs)
    in_maps = []
    for c in range(N_CORES):
        m = dict(arrs)
        m["xT"] = np.ascontiguousarray(xT[:, c * B_CORE : (c + 1) * B_CORE])
        in_maps.append(m)
    return in_maps


def kernel(**inputs):
    in_maps = make_in_maps(inputs)
    nc = build_nc()
    res = run_bass_kernel_spmd(nc, in_maps, list(range(N_CORES))).results
    return np.concatenate([r["y"] for r in res], axis=0)


# revision 22
# speedup vs baseline: 1.3811x; 1.3811x over previous
"""Trainium2 Bass kernel for nn_CIFAR10_Monarch_MLP2 (4-layer Monarch MLP + log_softmax).

Strategy
--------
Data-parallel over 8 NeuronCores: each core computes 2048 rows of the
16384-row batch with replicated weights; outputs are concatenated on host.

Per core, activations are kept feature-major ([feature partitions, batch
free]).  x is transposed to feature-major and cast to bf16 on the HOST, so
the device pipeline is load -> matmul with no casts or on-device transposes.
Batch tiles are NB=512 (full PSUM bank), halving instruction count vs 256.

The monarch permutation of layers 1/2 is folded into a host-side
re-arrangement of the weights:

 * w1 rows of block k are regrouped by destination plane l, each group padded
   to a fixed `chunk` (multiple of 32).  mm1 then runs natural M=128 tiles
   and its PSUM evictions scatter fragments into the plane layout with DVE
   copies, using the hardware's partition-shift capability (64-sized copies
   between halves, 32-sized between quadrants — HW-verified).
 * w2 columns are permuted to match the resulting plane-row order (pad rows
   get zero columns), so no data movement is needed for the permutation.

Layers 3/4 are small enough that their monarch pair (block-diag @ perm @
block-diag) is composed HOST-SIDE into a single dense matrix: PE cost is
instructions x N regardless of row/col utilization, and the dense form needs
fewer instructions (L3: 8 vs 12, L4: 4 vs 24) and no intermediate staging.
L4's bias is folded in as an extra contraction row (h4 row 127 := 1, W4c row
127 := bias), and its matmul is operand-swapped (activations stationary) so
logits land batch-major where log_softmax is a cheap free-dim reduction.
"""

import numpy as np
import ml_dtypes

import concourse.bass as bass
from concourse import bacc
import concourse.mybir as mybir
import concourse.tile as tile
from concourse.bass_utils import run_bass_kernel_spmd

F32 = mybir.dt.float32

# matmul operand dtype knob: mybir.dt.bfloat16 | float32r | float32
MM_DT = mybir.dt.bfloat16

N_CORES = 8
BATCH = 16384
B_CORE = BATCH // N_CORES  # 2048
NB = 512  # batch-tile free size (PSUM bank = 2KB -> 512 fp32)

# (p_dim, q_dim, chunk, s_dim) for monarch layers 1-2
LAYER_CFG = [
    (768, 750, 192, 750),
    (750, 250, 64, 250),
]


def _np_mmdt():
    return {
        mybir.dt.bfloat16: ml_dtypes.bfloat16,
        mybir.dt.float32r: np.float32,
        mybir.dt.float32: np.float32,
    }[MM_DT]


def arrange_layer(w1, w2, q_dim, chunk):
    """w1:(4,q,p), w2:(4,s,r=q) -> w1t:[4,p,QPAD] (mm1 lhsT), w2t:[4,QPAD,s]
    (mm2 lhsT), with the monarch permutation folded in (see module doc)."""
    nb, _, p_dim = w1.shape
    s_dim = w2.shape[1]
    QPAD = 4 * chunk
    w1t = np.zeros((nb, p_dim, QPAD), np.float32)
    w2t = np.zeros((nb, QPAD, s_dim), np.float32)
    for k in range(nb):
        for l in range(nb):
            qs = [q for q in range(q_dim) if (k * q_dim + q) % 4 == l]
            w1t[k, :, l * chunk : l * chunk + len(qs)] = w1[k, qs, :].T
            rs = [(k * q_dim + q) // 4 for q in qs]
            w2t[l, k * chunk : k * chunk + len(qs), :] = w2[l, :, rs]
    return w1t, w2t


def compose_monarch(w1, w2, out_features):
    """Dense [in_features, out_features] equivalent of one monarch linear."""
    nb, q_dim, p_dim = w1.shape
    _, s_dim, _ = w2.shape
    fin = nb * p_dim
    dense = np.zeros((fin, nb * s_dim), np.float64)
    w1d = w1.astype(np.float64)
    w2d = w2.astype(np.float64)
    for k in range(nb):
        for q in range(q_dim):
            f = k * q_dim + q
            l, r = f % 4, f // 4
            # out[l*s_dim + s] += w2[l, s, r] * (w1[k, q, :] . x[k*p_dim:...])
            dense[k * p_dim : (k + 1) * p_dim, l * s_dim : (l + 1) * s_dim] += (
                np.outer(w1d[k, q, :], w2d[l, :, r])
            )
    return dense[:, :out_features].astype(np.float32)


def evict_frags(k, m, chunk):
    """Fragments to scatter mm1's natural PSUM M-tile m of block k (padded
    rows [128m, 128m+128)) into the plane layout.

    Returns [(src_part0, size, plane_l, plane_tile, dst_part_base), ...].
    Fragment boundaries lie on the src 128-grid, dst 128-grid and l-chunk
    grid; shifted fragments are split to the DVE-legal 64 (or 32) grain.
    """
    grain = 64 if chunk % 64 == 0 else 32
    frags = []
    g = 128 * m
    end = 128 * (m + 1)
    while g < end:
        l = g // chunk
        dst = k * chunk + (g - l * chunk)  # global row within plane l
        # next boundary: chunk end, src tile end, dst tile end
        nb_ = min(end, (l + 1) * chunk, g + (128 - dst % 128))
        size = nb_ - g
        src_b = g - 128 * m
        dst_b = dst % 128
        if src_b % 128 == dst_b:
            frags.append((src_b, size, l, dst // 128, dst_b))
            g = nb_
        else:
            # shifted: emit at grain granularity (64: halves; 32: quadrants)
            step = min(grain, size)
            frags.append((src_b, step, l, dst // 128, dst_b))
            g += step
    return frags


def ktiles(p_dim):
    """[(row0, size), ...] 128-partition contraction tiles covering p_dim."""
    return [(r, min(128, p_dim - r)) for r in range(0, p_dim, 128)]


def prepare_weights(inputs):
    """Host-side arrangement of all weights/biases into DRAM-parameter arrays."""
    npdt = _np_mmdt()
    arrs = {}
    for li, (p_dim, q_dim, chunk, s_dim) in enumerate(LAYER_CFG, 1):
        w1 = np.asarray(inputs[f"w1_{li}"], np.float32)
        w2 = np.asarray(inputs[f"w2_{li}"], np.float32)
        w1t, w2t = arrange_layer(w1, w2, q_dim, chunk)
        arrs[f"w1t_{li}"] = w1t.astype(npdt)
        arrs[f"w2t_{li}"] = w2t.astype(npdt)
        bias = np.asarray(inputs[f"b{li}"], np.float32)  # [4*s_dim], f'=l*s+s
        # bias columns per (plane l, s-tile mt): [128, ncols]
        mts = ktiles(s_dim)
        cols = np.zeros((128, 4 * len(mts)), np.float32)
        for l in range(4):
            for mi, (m0, msz) in enumerate(mts):
                cols[:msz, l * len(mts) + mi] = bias[l * s_dim + m0 : l * s_dim + m0 + msz]
        arrs[f"bias_{li}"] = cols

    # ---- L3: dense composite [1000, 100], rows arranged to h3's tile layout
    w3c = compose_monarch(
        np.asarray(inputs["w1_3"], np.float32),
        np.asarray(inputs["w2_3"], np.float32), 100)
    h3_tiles = []  # (l2, m0, msz) in h3 storage order
    for l2 in range(4):
        for (m0, msz) in ktiles(250):
            h3_tiles.append((l2, m0, msz))
    # M padded to 128: rows 100..126 of h4 become relu(0)=0, and row 127
    # becomes relu(0 + bias=1) = 1 — the ones row for L4's folded bias.
    w3a = np.zeros((len(h3_tiles), 128, 128), np.float32)
    for ti, (l2, m0, msz) in enumerate(h3_tiles):
        w3a[ti, :msz, :100] = w3c[l2 * 250 + m0 : l2 * 250 + m0 + msz, :]
    arrs["w3a"] = w3a.astype(npdt)
    b3 = np.asarray(inputs["b3"], np.float32)
    b3c = np.pad(b3, (0, 28)).reshape(128, 1).astype(np.float32)
    b3c[127, 0] = 1.0
    arrs["bias_3"] = b3c

    # ---- L4: dense composite [100, 12] + bias folded at contraction row 127
    w4c = compose_monarch(
        np.asarray(inputs["w1_4"], np.float32),
        np.asarray(inputs["w2_4"], np.float32), 12)
    w4a = np.zeros((128, 12), np.float32)
    w4a[:100, :] = w4c
    b4 = np.asarray(inputs["b4"], np.float32)
    w4a[127, :10] = b4
    arrs["w4a"] = w4a.astype(npdt)
    return arrs


def build_nc(b_core=B_CORE, repeat=1, probe_mm1=False):
    """Build the single-core Bass program (SPMD: same program, per-core xT).
    repeat>1 re-runs the whole batch pipeline (for timing-by-differencing).
    probe_mm1 doubles L1 mm1's accumulation (identical result, pure extra PE
    work) to measure the marginal cost per matmul instruction."""
    nc = bacc.Bacc(None, target_bir_lowering=False)
    x_d = nc.declare_dram_parameter("xT", [3072, b_core], MM_DT, isOutput=False)
    y_d = nc.declare_dram_parameter("y", [b_core, 10], F32, isOutput=True)

    wd = {}
    for li, (p_dim, q_dim, chunk, s_dim) in enumerate(LAYER_CFG, 1):
        QPAD = 4 * chunk
        wd[f"w1t_{li}"] = nc.declare_dram_parameter(
            f"w1t_{li}", [4, p_dim, QPAD], MM_DT, isOutput=False)
        wd[f"w2t_{li}"] = nc.declare_dram_parameter(
            f"w2t_{li}", [4, QPAD, s_dim], MM_DT, isOutput=False)
        nmt = len(ktiles(s_dim))
        wd[f"bias_{li}"] = nc.declare_dram_parameter(
            f"bias_{li}", [128, 4 * nmt], F32, isOutput=False)
    wd["w3a"] = nc.declare_dram_parameter("w3a", [8, 128, 128], MM_DT, isOutput=False)
    wd["bias_3"] = nc.declare_dram_parameter("bias_3", [128, 1], F32, isOutput=False)
    wd["w4a"] = nc.declare_dram_parameter("w4a", [128, 12], MM_DT, isOutput=False)

    n_bt = b_core // NB

    with tile.TileContext(nc) as tc:
        with (
            tc.tile_pool(name="const", bufs=1) as const,
            tc.tile_pool(name="xT", bufs=8) as xTp,
            tc.tile_pool(name="acts", bufs=1) as acts,
            tc.tile_pool(name="h4p", bufs=1) as h4p,
            tc.tile_pool(name="psum_mm", bufs=7, space="PSUM") as psum_mm,
            tc.tile_pool(name="psum_s", bufs=1, space="PSUM") as psum_s,
            tc.tile_pool(name="sm", bufs=2) as smp,
        ):
            # ---- resident constants ----
            w1sb, w2sb, biassb = {}, {}, {}
            for li, (p_dim, q_dim, chunk, s_dim) in enumerate(LAYER_CFG, 1):
                QPAD = 4 * chunk
                kts = ktiles(p_dim)
                w1sb[li] = const.tile([128, len(kts) * 4 * QPAD], MM_DT, name=f"w1sb{li}", tag=f"w1sb{li}")
                for k in range(4):
                    for ki, (k0, ksz) in enumerate(kts):
                        col = (k * len(kts) + ki) * QPAD
                        nc.gpsimd.dma_start(
                            w1sb[li][:ksz, col : col + QPAD],
                            wd[f"w1t_{li}"][k, k0 : k0 + ksz, :],
                        )
                nrt = QPAD // 128
                w2sb[li] = const.tile([128, 4 * nrt * s_dim], MM_DT, name=f"w2sb{li}", tag=f"w2sb{li}")
                for l in range(4):
                    for rt in range(nrt):
                        col = (l * nrt + rt) * s_dim
                        # ACT HWDGE queue: streams in parallel with w1 on
                        # gpsimd so the first tile's mm2 isn't starved
                        nc.scalar.dma_start(
                            w2sb[li][:, col : col + s_dim],
                            wd[f"w2t_{li}"][l, 128 * rt : 128 * (rt + 1), :],
                        )
                nmt = len(ktiles(s_dim))
                biassb[li] = const.tile([128, 4 * nmt], F32, name=f"biassb{li}", tag=f"biassb{li}")
                nc.gpsimd.dma_start(biassb[li][:], wd[f"bias_{li}"][:, :])
            w3sb = const.tile([128, 8 * 128], MM_DT, name="w3sb", tag="w3sb")
            for ti in range(8):
                nc.gpsimd.dma_start(w3sb[:, ti * 128 : (ti + 1) * 128], wd["w3a"][ti, :, :])
            biassb3 = const.tile([128, 1], F32, name="biassb3", tag="biassb3")
            nc.gpsimd.dma_start(biassb3[:], wd["bias_3"][:, :])
            w4sb = const.tile([128, 12], MM_DT, name="w4sb", tag="w4sb")
            nc.gpsimd.dma_start(w4sb[:], wd["w4a"][:, :])

            # h4: [128, NB]; rewritten fully each batch-tile by the L3 evict
            # (rows 100..126 = 0, row 127 = 1 via the padded w3a/bias_3)
            h4 = h4p.tile([128, NB], MM_DT, name="h4", tag="h4")

            # ---- batch-tile pipeline ----
            for bt in [t for _ in range(repeat) for t in range(n_bt)]:
                # xT tiles: per block k, [128, 6*NB] feature-major bf16,
                # loaded straight from the host-transposed x.
                xk = []
                for k in range(4):
                    xt = xTp.tile([128, 6 * NB], MM_DT, name=f"x{k}", tag="xt")
                    src = x_d[768 * k : 768 * (k + 1), bt * NB : (bt + 1) * NB]
                    nc.sync.dma_start(
                        xt[:].rearrange("p (g c) -> p g c", g=6),
                        src.rearrange("(g p) c -> p g c", p=128),
                    )
                    xk.append(xt)
                h = None

                for li, (p_dim, q_dim, chunk, s_dim) in enumerate(LAYER_CFG, 1):
                    QPAD = 4 * chunk
                    ntl = QPAD // 128  # plane tiles
                    kts = ktiles(p_dim)
                    nkt = len(kts)
                    # --- mm1: natural block M-tiles (M=128, no col splits);
                    # evictions scatter to plane layout via (possibly
                    # partition-shifted) DVE fragment copies.  Legal shifts:
                    # any size at shift 0; 64-sized between halves; 32-sized
                    # between quadrants (HW-verified quadrant routing).
                    planes = acts.tile([128, 4 * ntl * NB], MM_DT, name=f"planes{li}", tag=f"planes{li}")
                    for k in range(4):
                        for m in range(ntl):
                            ps = psum_mm.tile([128, NB], F32, name="ps_mm", tag="ps_mm")
                            for rep in range(2 if (probe_mm1 and li == 1) else 1):
                                for ki, (k0, ksz) in enumerate(kts):
                                    if li == 1:
                                        rhs = xk[k][:, ki * NB : (ki + 1) * NB]
                                    else:
                                        hcol = in_tiles[k][ki][0]
                                        rhs = h[:ksz, hcol : hcol + NB]
                                    wcol = (k * nkt + ki) * QPAD + 128 * m
                                    nc.tensor.matmul(
                                        ps[:, :],
                                        w1sb[li][:ksz, wcol : wcol + 128],
                                        rhs,
                                        start=(ki == 0),
                                        stop=(ki == nkt - 1),
                                    )
                            for (s0, sz, l, jt, db) in evict_frags(k, m, chunk):
                                pcol = (l * ntl + jt) * NB
                                dst = planes[db : db + sz, pcol : pcol + NB]
                                src = ps[s0 : s0 + sz, :]
                                if s0 % 128 != db:
                                    # partition shift needs the DVE crossbar
                                    nc.vector.tensor_copy(dst, src)
                                else:
                                    # ACT (Copy shares the exp table set, so
                                    # no table-load thrash); keeps DVE free
                                    # for the shifted copies during mm1
                                    nc.scalar.copy(dst, src)

                    # --- mm2: planes -> next-layer blocks (relu+bias on evict)
                    mts = ktiles(s_dim)
                    nmt = len(mts)
                    hn = acts.tile([128, 4 * nmt * NB], MM_DT, name=f"h{li + 1}", tag=f"h{li + 1}")
                    for l in range(4):
                        for mi, (m0, msz) in enumerate(mts):
                            ps = psum_mm.tile([128, NB], F32, name="ps_mm", tag="ps_mm")
                            for rt in range(ntl):
                                wcol = (l * ntl + rt) * s_dim + m0
                                nc.tensor.matmul(
                                    ps[:msz, :],
                                    w2sb[li][:, wcol : wcol + msz],
                                    planes[:, (l * ntl + rt) * NB : (l * ntl + rt + 1) * NB],
                                    start=(rt == 0),
                                    stop=(rt == ntl - 1),
                                )
                            hcol = (l * nmt + mi) * NB
                            nc.scalar.activation(
                                hn[:msz, hcol : hcol + NB],
                                ps[:msz, :],
                                mybir.ActivationFunctionType.Relu,
                                bias=biassb[li][:msz, l * nmt + mi : l * nmt + mi + 1],
                            )
                    in_tiles = [
                        [((l * nmt + mi) * NB, msz) for mi, (m0, msz) in enumerate(mts)]
                        for l in range(4)
                    ]
                    h = hn

                # ---- L3: dense composite 1000 -> 100 (8 K-tiles, 1 M-tile)
                ps3 = psum_mm.tile([128, NB], F32, name="ps_mm", tag="ps_mm")
                for ti in range(8):
                    l2, ki = ti // 2, ti % 2
                    hcol, ksz = in_tiles[l2][ki]
                    nc.tensor.matmul(
                        ps3[:, :],
                        w3sb[:ksz, ti * 128 : ti * 128 + 128],
                        h[:ksz, hcol : hcol + NB],
                        start=(ti == 0),
                        stop=(ti == 7),
                    )
                nc.scalar.activation(
                    h4[:, :], ps3[:, :], mybir.ActivationFunctionType.Relu,
                    bias=biassb3[:, 0:1])

                # ---- L4: operand-swapped dense composite (bias via row 127):
                # logits[b, o] = sum_f h4[f, b] * w4a[f, o]
                for sub in range(NB // 128):
                    ps4 = psum_s.tile([128, 12], F32, name="ps4", tag="ps4")
                    nc.tensor.matmul(
                        ps4[:, :],
                        h4[:, sub * 128 : (sub + 1) * 128],
                        w4sb[:, :],
                        start=True,
                        stop=True,
                    )
                    # log_softmax over cols 0..9 (f32).  ln(s) is computed as
                    # bit-trick initial guess + one Newton step (2 ACT Exp ops)
                    # so the ACT engine only ever needs the exp table set —
                    # a Ln op would force a ~2.7us table-set switch per chain.
                    sm = smp.tile([128, 10], F32, name="sm", tag="sm")
                    nc.vector.tensor_copy(sm[:], ps4[:, 0:10])
                    mx = smp.tile([128, 1], F32, name="mx", tag="mx")
                    nc.vector.reduce_max(mx[:], sm[:], axis=mybir.AxisListType.X)
                    nmx = smp.tile([128, 1], F32, name="nmx", tag="nmx")
                    nc.vector.tensor_scalar_mul(nmx[:], mx[:], -1.0)
                    ex = smp.tile([128, 10], F32, name="ex", tag="ex")
                    nc.scalar.activation(
                        ex[:], sm[:], mybir.ActivationFunctionType.Exp, bias=nmx[:])
                    # post-exp chain on gpsimd (all SBUF-only ops) so DVE
                    # stays free for the next tile's mm1 evictions
                    sme = smp.tile([128, 1], F32, name="sme", tag="sme")
                    nc.vector.reduce_sum(sme[:], ex[:], axis=mybir.AxisListType.X)
                    # y0 = bits(s)*ln2/2^23 - (127*ln2 - 0.0298): |y0-ln s|<=.03
                    smi = smp.tile([128, 1], F32, name="smi", tag="smi")
                    nc.gpsimd.tensor_copy(smi[:], sme[:].bitcast(mybir.dt.int32))
                    y0 = smp.tile([128, 1], F32, name="y0", tag="y0")
                    nc.gpsimd.tensor_scalar(
                        y0[:], smi[:], 8.2629582e-8, 87.9998919,
                        mybir.AluOpType.mult, mybir.AluOpType.subtract)
                    e0 = smp.tile([128, 1], F32, name="e0", tag="e0")
                    nc.scalar.activation(
                        e0[:], y0[:], mybir.ActivationFunctionType.Exp,
                        bias=0.0, scale=-1.0)
                    # ofs = nmx - ln(s); ln(s) ~= y0 + s*exp(-y0) - 1
                    se = smp.tile([128, 1], F32, name="se", tag="se")
                    nc.gpsimd.tensor_mul(se[:], sme[:], e0[:])
                    t1 = smp.tile([128, 1], F32, name="t1", tag="t1")
                    nc.gpsimd.tensor_sub(t1[:], nmx[:], y0[:])
                    t2 = smp.tile([128, 1], F32, name="t2", tag="t2")
                    nc.gpsimd.tensor_sub(t2[:], t1[:], se[:])
                    ofs = smp.tile([128, 1], F32, name="ofs", tag="ofs")
                    nc.gpsimd.tensor_scalar_add(ofs[:], t2[:], 1.0)
                    out_t = smp.tile([128, 10], F32, name="out_t", tag="out_t")
                    nc.gpsimd.tensor_scalar_add(out_t[:], sm[:], ofs[:])
                    row0 = bt * NB + sub * 128
                    nc.sync.dma_start(y_d[row0 : row0 + 128, :], out_t[:])
    nc.compile()
    return nc


def make_in_maps(inputs):
    """Per-core input maps: host-transposed bf16 x shards + shared weights."""
    npdt = _np_mmdt()
    x = np.asarray(inputs["x"], np.float32)
    xT = x.T.astype(npdt)  # [3072, 16384] feature-major
    arrs = prepare_weights(inputs)
    in_maps = []
    for c in range(N_CORES):
        m = dict(arrs)
        m["xT"] = np.ascontiguousarray(xT[:, c * B_CORE : (c + 1) * B_CORE])
        in_maps.append(m)
    return in_maps


def kernel(**inputs):
    in_maps = make_in_maps(inputs)
    nc = build_nc()
    res = run_bass_kernel_spmd(nc, in_maps, list(range(N_CORES))).results
    return np.concatenate([r["y"] for r in res], axis=0)
